# revision 16
# baseline (speedup 1.0000x reference)
"""AudioQuantizer (dual-codebook VQ) Trainium2 kernel.

Full inputs -> full outputs; internally data-parallel over batch B=8 across
8 NeuronCores (one batch row per core). Per core, for each quantizer q and
codebook cb:

  scores[t, k] = <x[t, q*128:(q+1)*128], cb[q, k, :]>   (fp16 hi/lo 3-term
                  matmuls, fp32 PSUM accumulate -> ~fp32-exact)
  idx[t]       = argmax_k(scores[t, k] - 0.5*||cb[q, k]||^2)
                  (single-pass fused custom DVE op, per-partition scan)
  quant[t]     = cb[q, idx[t]]   (SWDGE indirect gather from HBM)

Self-contained: includes the walrus-compat wait-splitting pass and the
custom DVE argmax op registration.
"""

import sys

if "/opt/trn_rl_repo" not in sys.path:
    sys.path.insert(0, "/opt/trn_rl_repo")

import numpy as np

import concourse.bass as bass
import concourse.mybir as mybir
from concourse.bass import IndirectOffsetOnAxis
from concourse.bass_utils import run_bass_kernel_spmd
from concourse.library_overlay import lower_extended_insts
from concourse.tile import TileContext

# ---------------------------------------------------------------------------
# problem constants (hardcoded per spec)
B, T, D = 8, 2048, 1024
Q, K, DS = 8, 1024, 128
NCORES = 8
TT = T // 128  # 16 t-tiles per core
F32 = mybir.dt.float32
F16 = mybir.dt.float16
I32 = mybir.dt.int32

# ---------------------------------------------------------------------------
# walrus-compat: split >1 sync wait per instruction into EventSemaphore
# carriers (this walrus accepts 1 wait per inst, 2 on EventSemaphore).
_ev_counter = [0]


def _make_event_sem(engine, waits):
    _ev_counter[0] += 1
    ev = mybir.InstEventSemaphore(name=f"EVWFIX-{_ev_counter[0]}", ins=[], outs=[])
    ev.engine = engine
    ev.sync_info = mybir.SyncInfo(on_wait=list(waits), on_update=[])
    return ev


def _split_excess_waits(nc):
    for f in nc.m.functions:
        for b in f.blocks:
            insts = list(b.instructions)
            out = []
            changed = False
            for inst in insts:
                si = inst.sync_info
                waits = list(si.on_wait) if si and si.on_wait else []
                cap = 2 if inst.opcode == "EventSemaphore" else 1
                if len(waits) > cap:
                    extra = waits[: len(waits) - cap]
                    for i in range(0, len(extra), 2):
                        out.append(_make_event_sem(inst.engine, extra[i : i + 2]))
                    si.on_wait = waits[len(waits) - cap :]
                    changed = True
                out.append(inst)
            if changed:
                b.instructions = out


def _finalize(nc):
    lower_extended_insts(nc)
    _split_excess_waits(nc)


# ---------------------------------------------------------------------------
# custom DVE op: accum_out[p] = argmax_k (in0[p,k] - in1[p,k]) in one pass
def _register_argmax_op():
    from concourse.dve_spec import (
        AluOp,
        Idx,
        MaxNeg,
        Spec,
        Src0,
        Src1,
        eq,
        lower,
        maxx,
        scan,
        select,
    )
    from concourse.dve_ops import DveOp, OPS, CUSTOM_DVE_SPECS, _SUB_OPCODE_FOR_NAME
    from concourse.dve_uop import DveOpSpec

    name = "ARGMAX_BIAS_ANT"
    for op in OPS:
        if op.name == name:
            return op

    def _ref(in0, in1, c0, c1, c2):
        u = in0.astype(np.float32) - in1.astype(np.float32)
        r = np.maximum.accumulate(u, axis=-1)
        idxv = np.arange(u.shape[-1], dtype=np.float32)
        body = np.where(u == r, idxv, np.float32(-3.4028235e38))
        acc = body.reshape(body.shape[0], -1).max(axis=-1, keepdims=True)
        return body, acc

    u = Src0 - Src1
    r = scan(AluOp.MAX, u)
    spec = Spec(body=select(eq(u, r), Idx, MaxNeg), accum=maxx, reference=_ref)
    row = max(_SUB_OPCODE_FOR_NAME.values()) + 1
    assert row < 0x20
    _SUB_OPCODE_FOR_NAME[name] = row
    shas = {}
    for ver in ("v3", "v4"):
        shas[ver] = DveOpSpec(
            name=name, opcode=row, uops=lower(spec, ver=ver), rd1_en=True
        ).sha(ver)
    op = DveOp(name, spec, subdim=False, uops_sha=shas)
    OPS.append(op)
    CUSTOM_DVE_SPECS[name] = spec
    return op


# ---------------------------------------------------------------------------
def build_nc():
    """One-core program; run SPMD on 8 cores with per-core input shards."""
    argmax = _register_argmax_op()
    nc = bass.Bass()

    # fp16 hi/lo split of transposed x: [q, d=128, t=2048]
    xh_ext = nc.declare_dram_parameter("xh", [Q, DS, T], F16, isOutput=False)
    xl_ext = nc.declare_dram_parameter("xl", [Q, DS, T], F16, isOutput=False)
    # fp16 hi/lo split of transposed codebooks: [cb, q, d=128, k=1024]
    cbh_ext = nc.declare_dram_parameter("cbh", [2, Q, DS, K], F16, isOutput=False)
    cbl_ext = nc.declare_dram_parameter("cbl", [2, Q, DS, K], F16, isOutput=False)
    # 0.5*||c||^2 bias, replicated across 128 partitions: [cb, q, 128, k]
    c2_ext = nc.declare_dram_parameter("c2", [2, Q, 128, K], F32, isOutput=False)
    # gather source: [cb*q*K, DS] fp32 rows
    cbg_ext = nc.declare_dram_parameter("cbg", [2 * Q * K, DS], F32, isOutput=False)

    outs = {}
    for cbn in ("t", "d"):
        outs[f"quant_{cbn}"] = nc.declare_dram_parameter(
            f"quant_{cbn}", [T, D], F32, isOutput=True
        )
        outs[f"idx_{cbn}"] = nc.declare_dram_parameter(
            f"idx_{cbn}", [T, Q], I32, isOutput=True
        )

    with TileContext(nc) as tc:
        with (
            tc.tile_pool(name="const", bufs=1) as constp,
            tc.tile_pool(name="cbw", bufs=2) as cbwp,
            tc.tile_pool(name="c2p", bufs=2) as c2p,
            tc.tile_pool(name="xp", bufs=3) as xp,
            tc.tile_pool(name="junkp", bufs=2) as junkp,
            tc.tile_pool(name="idxp", bufs=1) as idxp,
            tc.tile_pool(name="gp", bufs=4) as gp,
            tc.tile_pool(name="ps_sc", bufs=4, space="PSUM") as ps_sc,
        ):
            # argmax accumulators, parity-double-buffered over q so that the
            # next q's writes never WAR against this q's gather reads
            idxf = [
                [
                    idxp.tile([128, TT], F32, tag=f"idxf{c}_{p}", name=f"idxf{c}_{p}")
                    for p in range(2)
                ]
                for c in range(2)
            ]
            idxi = [
                [
                    idxp.tile([128, TT], I32, tag=f"idxi{c}_{p}", name=f"idxi{c}_{p}")
                    for p in range(2)
                ]
                for c in range(2)
            ]
            # persistent per-cb idx output assembly [128, tt, q]
            idxo = [
                idxp.tile([128, TT, Q], I32, tag=f"idxo{c}", name=f"idxo{c}")
                for c in range(2)
            ]

            for q in range(Q):
                cbh = [cbwp.tile([DS, K], F16, tag=f"cbh{c}", name=f"cbh{c}_{q}") for c in range(2)]
                cbl = [cbwp.tile([DS, K], F16, tag=f"cbl{c}", name=f"cbl{c}_{q}") for c in range(2)]
                c2b = [c2p.tile([128, K], F32, tag=f"c2b{c}", name=f"c2b{c}_{q}") for c in range(2)]
                xh_q = xp.tile([DS, T], F16, tag="xh_q")
                xl_q = xp.tile([DS, T], F16, tag="xl_q")
                nc.sync.dma_start(out=cbh[0][:], in_=cbh_ext[0, q])
                nc.sync.dma_start(out=xh_q[:], in_=xh_ext[q])
                nc.sync.dma_start(out=xl_q[:], in_=xl_ext[q])
                nc.sync.dma_start(out=cbl[0][:], in_=cbl_ext[0, q])
                nc.sync.dma_start(out=cbh[1][:], in_=cbh_ext[1, q])
                nc.sync.dma_start(out=cbl[1][:], in_=cbl_ext[1, q])
                nc.sync.dma_start(out=c2b[0][:], in_=c2_ext[0, q])
                nc.sync.dma_start(out=c2b[1][:], in_=c2_ext[1, q])

                for tt in range(TT):
                    tsl = slice(tt * 128, (tt + 1) * 128)
                    xh = xh_q[:, tsl]
                    xl = xl_q[:, tsl]

                    scores = [
                        ps_sc.tile([128, K], F32, tag="scores", name=f"sc{q}_{tt}_{c2i}")
                        for c2i in range(2)
                    ]
                    # 3-term accumulation: xh@ch (start), xh@cl, xl@ch (stop)
                    for kc in range(2):
                        sl = slice(kc * 512, (kc + 1) * 512)
                        for c in range(2):
                            nc.tensor.matmul(
                                scores[c][:, sl], xh, cbh[c][:, sl],
                                start=True, stop=False,
                            )
                    for kc in range(2):
                        sl = slice(kc * 512, (kc + 1) * 512)
                        for c in range(2):
                            nc.tensor.matmul(
                                scores[c][:, sl], xh, cbl[c][:, sl],
                                start=False, stop=False,
                            )
                    for kc in range(2):
                        sl = slice(kc * 512, (kc + 1) * 512)
                        for c in range(2):
                            nc.tensor.matmul(
                                scores[c][:, sl], xl, cbh[c][:, sl],
                                start=False, stop=True,
                            )

                    for c in range(2):
                        nc.vector._custom_dve(
                            argmax,
                            out=scores[c][:],
                            in0=scores[c][:],
                            in1=c2b[c][:],
                            accum_out=idxf[c][q % 2][:, tt : tt + 1],
                        )

                # gather + outputs for this q; idx cast in 4-tile chunks on
                # ACT so gathers start while later tiles still compute
                gtiles = {}
                for c, cbn in enumerate(("t", "d")):
                    par = q % 2
                    gtiles[c] = gp.tile([128, TT, DS], F32, tag="g", name=f"g{q}_{c}")
                    nc.scalar.copy(
                        out=idxo[c][:, :, q : q + 1], in_=idxf[c][par][:, :, None]
                    )
                chunks = [(0, 2), (2, 4), (4, 8), (8, 12), (12, 16)]
                for c0, c1 in chunks:
                    csl = slice(c0, c1)
                    for c in range(2):
                        par = q % 2
                        nc.vector.tensor_copy(
                            idxi[c][par][:, csl], idxf[c][par][:, csl]
                        )
                        for tt in range(c0, c1):
                            nc.gpsimd.indirect_dma_start(
                                out=gtiles[c][:, tt, :],
                                out_offset=None,
                                in_=cbg_ext[:],
                                in_offset=IndirectOffsetOnAxis(
                                    ap=idxi[c][par][:, tt : tt + 1], axis=0
                                ),
                                element_offset=(c * Q + q) * K * DS,
                            )
                for c, cbn in enumerate(("t", "d")):
                    dst = outs[f"quant_{cbn}"][:, q * DS : (q + 1) * DS].rearrange(
                        "(j p) d -> p j d", p=128
                    )
                    nc.sync.dma_start(out=dst[:, :8], in_=gtiles[c][:, :8])
                    nc.sync.dma_start(out=dst[:, 8:], in_=gtiles[c][:, 8:])

            for c, cbn in enumerate(("t", "d")):
                nc.scalar.dma_start(
                    out=outs[f"idx_{cbn}"][:, :].rearrange("(j p) q -> p j q", p=128),
                    in_=idxo[c][:],
                )

    _finalize(nc)
    return nc


_NC_CACHE = None


def _get_nc():
    global _NC_CACHE
    if _NC_CACHE is None:
        _NC_CACHE = build_nc()
    return _NC_CACHE


def _prep_x(x):
    # [B, T, Q, DS] -> [B, Q, DS, T] fp16 hi/lo
    xq = np.transpose(x.reshape(B, T, Q, DS), (0, 2, 3, 1))
    xh = np.ascontiguousarray(xq, dtype=np.float16)
    xl = (xq - xh.astype(np.float32)).astype(np.float16)
    return xh, xl


def _prep_host(temporal_cb, depth_cb):
    cbs = np.stack([temporal_cb, depth_cb]).astype(np.float32)  # [2, Q, K, DS]
    cbT = np.transpose(cbs, (0, 1, 3, 2))  # [2, Q, DS, K]
    cbh = cbT.astype(np.float16)
    cbl = (cbT - cbh.astype(np.float32)).astype(np.float16)
    c2 = 0.5 * (cbs.astype(np.float64) ** 2).sum(-1).astype(np.float32)  # [2,Q,K]
    c2rep = np.broadcast_to(c2[:, :, None, :], (2, Q, 128, K)).copy()
    cbg = cbs.reshape(2 * Q * K, DS)
    return cbh, cbl, c2rep, cbg


def kernel(x, temporal_cb, depth_cb):
    x = np.asarray(x, dtype=np.float32)
    temporal_cb = np.asarray(temporal_cb, dtype=np.float32)
    depth_cb = np.asarray(depth_cb, dtype=np.float32)

    cbh, cbl, c2rep, cbg = _prep_host(temporal_cb, depth_cb)
    xh, xl = _prep_x(x)
    nc = _get_nc()

    in_maps = []
    for b in range(NCORES):
        in_maps.append(
            {
                "xh": xh[b],
                "xl": xl[b],
                "cbh": cbh,
                "cbl": cbl,
                "c2": c2rep,
                "cbg": cbg,
            }
        )
    res = run_bass_kernel_spmd(nc, in_maps, list(range(NCORES)))

    quant_t = np.stack([res.results[b]["quant_t"] for b in range(NCORES)])
    idx_t = np.stack([res.results[b]["idx_t"] for b in range(NCORES)]).astype(np.int32)
    quant_d = np.stack([res.results[b]["quant_d"] for b in range(NCORES)])
    idx_d = np.stack([res.results[b]["idx_d"] for b in range(NCORES)]).astype(np.int32)
    return quant_t, idx_t, quant_d, idx_d


# revision 17
# speedup vs baseline: 1.0226x; 1.0226x over previous
"""AudioQuantizer (dual-codebook VQ) Trainium2 kernel.

Full inputs -> full outputs; internally data-parallel over batch B=8 across
8 NeuronCores (one batch row per core). Per core, for each quantizer q and
codebook cb:

  scores[t, k] = <x[t, q*128:(q+1)*128], cb[q, k, :]>   (fp16 hi/lo 3-term
                  matmuls, fp32 PSUM accumulate -> ~fp32-exact)
  idx[t]       = argmax_k(scores[t, k] - 0.5*||cb[q, k]||^2)
                  (single-pass fused custom DVE op, per-partition scan)
  quant[t]     = cb[q, idx[t]]   (SWDGE indirect gather from HBM)

Self-contained: includes the walrus-compat wait-splitting pass and the
custom DVE argmax op registration.
"""

import sys

if "/opt/trn_rl_repo" not in sys.path:
    sys.path.insert(0, "/opt/trn_rl_repo")

import numpy as np

import concourse.bass as bass
import concourse.mybir as mybir
from concourse.bass import IndirectOffsetOnAxis
from concourse.bass_utils import run_bass_kernel_spmd
from concourse.library_overlay import lower_extended_insts
from concourse.tile import TileContext

# ---------------------------------------------------------------------------
# problem constants (hardcoded per spec)
B, T, D = 8, 2048, 1024
Q, K, DS = 8, 1024, 128
NCORES = 8
TT = T // 128  # 16 t-tiles per core
F32 = mybir.dt.float32
F16 = mybir.dt.float16
I32 = mybir.dt.int32

# ---------------------------------------------------------------------------
# walrus-compat: split >1 sync wait per instruction into EventSemaphore
# carriers (this walrus accepts 1 wait per inst, 2 on EventSemaphore).
_ev_counter = [0]


def _make_event_sem(engine, waits):
    _ev_counter[0] += 1
    ev = mybir.InstEventSemaphore(name=f"EVWFIX-{_ev_counter[0]}", ins=[], outs=[])
    ev.engine = engine
    ev.sync_info = mybir.SyncInfo(on_wait=list(waits), on_update=[])
    return ev


def _split_excess_waits(nc):
    for f in nc.m.functions:
        for b in f.blocks:
            insts = list(b.instructions)
            out = []
            changed = False
            for inst in insts:
                si = inst.sync_info
                waits = list(si.on_wait) if si and si.on_wait else []
                cap = 2 if inst.opcode == "EventSemaphore" else 1
                if len(waits) > cap:
                    extra = waits[: len(waits) - cap]
                    for i in range(0, len(extra), 2):
                        out.append(_make_event_sem(inst.engine, extra[i : i + 2]))
                    si.on_wait = waits[len(waits) - cap :]
                    changed = True
                out.append(inst)
            if changed:
                b.instructions = out


def _finalize(nc):
    lower_extended_insts(nc)
    _split_excess_waits(nc)


# ---------------------------------------------------------------------------
# custom DVE op: accum_out[p] = argmax_k (in0[p,k] - in1[p,k]) in one pass
def _register_argmax_op():
    from concourse.dve_spec import (
        AluOp,
        Idx,
        MaxNeg,
        Spec,
        Src0,
        Src1,
        eq,
        lower,
        maxx,
        scan,
        select,
    )
    from concourse.dve_ops import DveOp, OPS, CUSTOM_DVE_SPECS, _SUB_OPCODE_FOR_NAME
    from concourse.dve_uop import DveOpSpec

    name = "ARGMAX_BIAS_ANT"
    for op in OPS:
        if op.name == name:
            return op

    def _ref(in0, in1, c0, c1, c2):
        u = in0.astype(np.float32) - in1.astype(np.float32)
        r = np.maximum.accumulate(u, axis=-1)
        idxv = np.arange(u.shape[-1], dtype=np.float32)
        body = np.where(u == r, idxv, np.float32(-3.4028235e38))
        acc = body.reshape(body.shape[0], -1).max(axis=-1, keepdims=True)
        return body, acc

    u = Src0 - Src1
    r = scan(AluOp.MAX, u)
    spec = Spec(body=select(eq(u, r), Idx, MaxNeg), accum=maxx, reference=_ref)
    row = max(_SUB_OPCODE_FOR_NAME.values()) + 1
    assert row < 0x20
    _SUB_OPCODE_FOR_NAME[name] = row
    shas = {}
    for ver in ("v3", "v4"):
        shas[ver] = DveOpSpec(
            name=name, opcode=row, uops=lower(spec, ver=ver), rd1_en=True
        ).sha(ver)
    op = DveOp(name, spec, subdim=False, uops_sha=shas)
    OPS.append(op)
    CUSTOM_DVE_SPECS[name] = spec
    return op


# ---------------------------------------------------------------------------
def build_nc():
    """One-core program; run SPMD on 8 cores with per-core input shards."""
    argmax = _register_argmax_op()
    nc = bass.Bass()

    # fp16 hi/lo split of transposed x: [q, d=128, t=2048]
    xh_ext = nc.declare_dram_parameter("xh", [Q, DS, T], F16, isOutput=False)
    xl_ext = nc.declare_dram_parameter("xl", [Q, DS, T], F16, isOutput=False)
    # fp16 hi/lo split of transposed codebooks: [cb, q, d=128, k=1024]
    cbh_ext = nc.declare_dram_parameter("cbh", [2, Q, DS, K], F16, isOutput=False)
    cbl_ext = nc.declare_dram_parameter("cbl", [2, Q, DS, K], F16, isOutput=False)
    # 0.5*||c||^2 bias, replicated across 128 partitions: [cb, q, 128, k]
    c2_ext = nc.declare_dram_parameter("c2", [2, Q, 128, K], F32, isOutput=False)
    # gather source: [cb*q*K, DS] fp32 rows
    cbg_ext = nc.declare_dram_parameter("cbg", [2 * Q * K, DS], F32, isOutput=False)

    outs = {}
    for cbn in ("t", "d"):
        outs[f"quant_{cbn}"] = nc.declare_dram_parameter(
            f"quant_{cbn}", [T, D], F32, isOutput=True
        )
        outs[f"idx_{cbn}"] = nc.declare_dram_parameter(
            f"idx_{cbn}", [T, Q], I32, isOutput=True
        )

    with TileContext(nc) as tc:
        with (
            tc.tile_pool(name="const", bufs=1) as constp,
            tc.tile_pool(name="cbw", bufs=2) as cbwp,
            tc.tile_pool(name="c2p", bufs=2) as c2p,
            tc.tile_pool(name="xp", bufs=3) as xp,
            tc.tile_pool(name="junkp", bufs=2) as junkp,
            tc.tile_pool(name="idxp", bufs=1) as idxp,
            tc.tile_pool(name="gp", bufs=4) as gp,
            tc.tile_pool(name="ps_sc", bufs=4, space="PSUM") as ps_sc,
        ):
            # argmax accumulators, parity-double-buffered over q so that the
            # next q's writes never WAR against this q's gather reads
            idxf = [
                [
                    idxp.tile([128, TT], F32, tag=f"idxf{c}_{p}", name=f"idxf{c}_{p}")
                    for p in range(2)
                ]
                for c in range(2)
            ]
            idxi = [
                [
                    idxp.tile([128, TT], I32, tag=f"idxi{c}_{p}", name=f"idxi{c}_{p}")
                    for p in range(2)
                ]
                for c in range(2)
            ]
            # persistent per-cb idx output assembly [128, tt, q]
            idxo = [
                idxp.tile([128, TT, Q], I32, tag=f"idxo{c}", name=f"idxo{c}")
                for c in range(2)
            ]

            for q in range(Q):
                cbh = [cbwp.tile([DS, K], F16, tag=f"cbh{c}", name=f"cbh{c}_{q}") for c in range(2)]
                cbl = [cbwp.tile([DS, K], F16, tag=f"cbl{c}", name=f"cbl{c}_{q}") for c in range(2)]
                c2b = [c2p.tile([128, K], F32, tag=f"c2b{c}", name=f"c2b{c}_{q}") for c in range(2)]
                xh_q = xp.tile([DS, T], F16, tag="xh_q")
                xl_q = xp.tile([DS, T], F16, tag="xl_q")
                nc.sync.dma_start(out=cbh[0][:], in_=cbh_ext[0, q])
                nc.sync.dma_start(out=xh_q[:], in_=xh_ext[q])
                nc.sync.dma_start(out=xl_q[:], in_=xl_ext[q])
                nc.sync.dma_start(out=cbl[0][:], in_=cbl_ext[0, q])
                nc.sync.dma_start(out=cbh[1][:], in_=cbh_ext[1, q])
                nc.sync.dma_start(out=cbl[1][:], in_=cbl_ext[1, q])
                nc.sync.dma_start(out=c2b[0][:], in_=c2_ext[0, q])
                nc.sync.dma_start(out=c2b[1][:], in_=c2_ext[1, q])

                gtiles = {}
                for c in range(2):
                    gtiles[c] = gp.tile(
                        [128, TT, DS], F32, tag="g", name=f"g{q}_{c}"
                    )
                chunk_after = {1: (0, 2), 3: (2, 4), 7: (4, 8), 11: (8, 12), 15: (12, 16)}

                for tt in range(TT):
                    tsl = slice(tt * 128, (tt + 1) * 128)
                    xh = xh_q[:, tsl]
                    xl = xl_q[:, tsl]

                    scores = [
                        ps_sc.tile([128, K], F32, tag="scores", name=f"sc{q}_{tt}_{c2i}")
                        for c2i in range(2)
                    ]
                    # 3-term accumulation: xh@ch (start), xh@cl, xl@ch (stop)
                    for kc in range(2):
                        sl = slice(kc * 512, (kc + 1) * 512)
                        for c in range(2):
                            nc.tensor.matmul(
                                scores[c][:, sl], xh, cbh[c][:, sl],
                                start=True, stop=False,
                            )
                    for kc in range(2):
                        sl = slice(kc * 512, (kc + 1) * 512)
                        for c in range(2):
                            nc.tensor.matmul(
                                scores[c][:, sl], xh, cbl[c][:, sl],
                                start=False, stop=False,
                            )
                    for kc in range(2):
                        sl = slice(kc * 512, (kc + 1) * 512)
                        for c in range(2):
                            nc.tensor.matmul(
                                scores[c][:, sl], xl, cbh[c][:, sl],
                                start=False, stop=True,
                            )

                    for c in range(2):
                        nc.vector._custom_dve(
                            argmax,
                            out=scores[c][:],
                            in0=scores[c][:],
                            in1=c2b[c][:],
                            accum_out=idxf[c][q % 2][:, tt : tt + 1],
                        )

                    if tt in chunk_after:
                        c0, c1 = chunk_after[tt]
                        csl = slice(c0, c1)
                        for c in range(2):
                            par = q % 2
                            nc.vector.tensor_copy(
                                idxi[c][par][:, csl], idxf[c][par][:, csl]
                            )
                            for gt in range(c0, c1):
                                nc.gpsimd.indirect_dma_start(
                                    out=gtiles[c][:, gt, :],
                                    out_offset=None,
                                    in_=cbg_ext[:],
                                    in_offset=IndirectOffsetOnAxis(
                                        ap=idxi[c][par][:, gt : gt + 1], axis=0
                                    ),
                                    element_offset=(c * Q + q) * K * DS,
                                )

                for c, cbn in enumerate(("t", "d")):
                    nc.scalar.copy(
                        out=idxo[c][:, :, q : q + 1],
                        in_=idxf[c][q % 2][:, :, None],
                    )
                for c, cbn in enumerate(("t", "d")):
                    dst = outs[f"quant_{cbn}"][:, q * DS : (q + 1) * DS].rearrange(
                        "(j p) d -> p j d", p=128
                    )
                    nc.sync.dma_start(out=dst[:, :8], in_=gtiles[c][:, :8])
                    nc.sync.dma_start(out=dst[:, 8:], in_=gtiles[c][:, 8:])

            for c, cbn in enumerate(("t", "d")):
                nc.scalar.dma_start(
                    out=outs[f"idx_{cbn}"][:, :].rearrange("(j p) q -> p j q", p=128),
                    in_=idxo[c][:],
                )

    _finalize(nc)
    return nc


_NC_CACHE = None


def _get_nc():
    global _NC_CACHE
    if _NC_CACHE is None:
        _NC_CACHE = build_nc()
    return _NC_CACHE


def _prep_x(x):
    # [B, T, Q, DS] -> [B, Q, DS, T] fp16 hi/lo
    xq = np.transpose(x.reshape(B, T, Q, DS), (0, 2, 3, 1))
    xh = np.ascontiguousarray(xq, dtype=np.float16)
    xl = (xq - xh.astype(np.float32)).astype(np.float16)
    return xh, xl


def _prep_host(temporal_cb, depth_cb):
    cbs = np.stack([temporal_cb, depth_cb]).astype(np.float32)  # [2, Q, K, DS]
    cbT = np.transpose(cbs, (0, 1, 3, 2))  # [2, Q, DS, K]
    cbh = cbT.astype(np.float16)
    cbl = (cbT - cbh.astype(np.float32)).astype(np.float16)
    c2 = 0.5 * (cbs.astype(np.float64) ** 2).sum(-1).astype(np.float32)  # [2,Q,K]
    c2rep = np.broadcast_to(c2[:, :, None, :], (2, Q, 128, K)).copy()
    cbg = cbs.reshape(2 * Q * K, DS)
    return cbh, cbl, c2rep, cbg


def kernel(x, temporal_cb, depth_cb):
    x = np.asarray(x, dtype=np.float32)
    temporal_cb = np.asarray(temporal_cb, dtype=np.float32)
    depth_cb = np.asarray(depth_cb, dtype=np.float32)

    cbh, cbl, c2rep, cbg = _prep_host(temporal_cb, depth_cb)
    xh, xl = _prep_x(x)
    nc = _get_nc()

    in_maps = []
    for b in range(NCORES):
        in_maps.append(
            {
                "xh": xh[b],
                "xl": xl[b],
                "cbh": cbh,
                "cbl": cbl,
                "c2": c2rep,
                "cbg": cbg,
            }
        )
    res = run_bass_kernel_spmd(nc, in_maps, list(range(NCORES)))

    quant_t = np.stack([res.results[b]["quant_t"] for b in range(NCORES)])
    idx_t = np.stack([res.results[b]["idx_t"] for b in range(NCORES)]).astype(np.int32)
    quant_d = np.stack([res.results[b]["quant_d"] for b in range(NCORES)])
    idx_d = np.stack([res.results[b]["idx_d"] for b in range(NCORES)]).astype(np.int32)
    return quant_t, idx_t, quant_d, idx_d
